# revision 1
# baseline (speedup 1.0000x reference)
"""GAT NodeEncoder kernel for Trainium2 (8 NeuronCores, data-parallel over batch).

Reference computation (per batch element b, per node n):
    src  = E[subgraph[b,n]];  nei_i = E[neighs[b,n,i]]
    s_0  = leaky(src@a1 + src@a2 + a_b); s_i = leaky(src@a1 + nei_i@a2 + a_b) + mask_i*-1e9
    att  = softmax(s); v = sum_i att_i * emb_i
    x = leaky(fc1 @ [v; local_stats; gstat] + b1); out = leaky(fc2 @ x + b2)

Sharding: batch B=8 over 8 cores (1 batch row / core), emb table replicated.

Key performance constraint: the indirect-DMA gather costs ~1.1us per call and
each call moves at most 128 rows (one per SBUF partition).  So the kernel
minimizes gathered rows:
  - masked neighbors are dropped entirely (exact: exp(-1e9 + x) == 0 in f32),
  - nodes are sorted by surviving-degree so each 128-node tile has a uniform
    slot count c_t (per-node padding only up to the tile max),
  - the src row is gathered once; its a1-dot is computed separately on DVE.
Outputs are scatter-stored back to original node order via indirect DMA.

Engine split per tile: gather (gpsimd SWDGE) | scores = one bf16 mul +
segmented reduce (DVE) | softmax smalls (DVE+ACT, fused exp+sum) | attention
prescale (DVE tensor_scalar x c_t) | weighted sum = PE transpose-accumulate
vs identity | fc1/fc2 on PE (host-pretransposed weights, bias via PE ones-
trick) | activations + PSUM moves on ACT.
"""

import os
from contextlib import ExitStack

import numpy as np
import ml_dtypes

import concourse.bass as bass
import concourse.bacc as bacc
import concourse.tile as tile
from concourse import mybir
from concourse import bass_utils

B, S, N, H, NLS = 8, 1024, 32, 128, 4
NUM_NODES = 100001
TILE = 128
NT = S // TILE
F32 = mybir.dt.float32
BF16 = mybir.dt.bfloat16
I32 = mybir.dt.int32
AF = mybir.ActivationFunctionType
ALU = mybir.AluOpType

_cached = {}


def _build_program(slots, reps=1):
    """slots: per-tile slot counts (len NT tuple), slot 0 = src row."""
    nt = len(slots)
    S_ = nt * TILE
    ctot = int(sum(slots))
    offs = np.concatenate([[0], np.cumsum(slots)]).astype(int)

    nc = bacc.Bacc(target_bir_lowering=False, debug=False, enable_asserts=False)

    emb = nc.dram_tensor("emb", [NUM_NODES, H], F32, kind="ExternalInput")
    idx = nc.dram_tensor("idx", [TILE, ctot], I32, kind="ExternalInput")
    padm = nc.dram_tensor("padm", [TILE, ctot], F32, kind="ExternalInput")
    outrow = nc.dram_tensor("outrow", [TILE, nt], I32, kind="ExternalInput")
    statst = nc.dram_tensor("statst", [NLS + 1, S_], BF16, kind="ExternalInput")
    a2rep_d = nc.dram_tensor("a2rep", [1, H], BF16, kind="ExternalInput")
    a1rep_d = nc.dram_tensor("a1rep", [1, H], BF16, kind="ExternalInput")
    ab_rep = nc.dram_tensor("ab_rep", [TILE, 1], F32, kind="ExternalInput")
    ident = nc.dram_tensor("ident", [TILE, TILE], BF16, kind="ExternalInput")
    w1t_a = nc.dram_tensor("w1t_a", [H, H], BF16, kind="ExternalInput")
    w1t_b = nc.dram_tensor("w1t_b", [NLS + 1, H], BF16, kind="ExternalInput")
    b1 = nc.dram_tensor("b1", [H, 1], F32, kind="ExternalInput")
    w2t = nc.dram_tensor("w2t", [H, H], BF16, kind="ExternalInput")
    b2row = nc.dram_tensor("b2row", [1, H], BF16, kind="ExternalInput")
    onesc = nc.dram_tensor("onesc", [1, TILE], BF16, kind="ExternalInput")
    out = nc.dram_tensor("out", [S, H], F32, kind="ExternalOutput")

    cmax = int(max(slots))

    with tile.TileContext(nc) as tc, ExitStack() as ctx:
        const = ctx.enter_context(tc.tile_pool(name="const", bufs=1))
        gpool = ctx.enter_context(tc.tile_pool(name="gpool", bufs=2))
        spool = ctx.enter_context(tc.tile_pool(name="spool", bufs=2))
        small = ctx.enter_context(tc.tile_pool(name="small", bufs=4))
        opool = ctx.enter_context(tc.tile_pool(name="opool", bufs=2))
        psum = ctx.enter_context(tc.tile_pool(name="psum", bufs=2, space="PSUM"))

        # ---- constants ----
        c_idx0 = const.tile([TILE, ctot], I32)
        nc.sync.dma_start(out=c_idx0[:], in_=idx[:, :])
        c_padm0 = const.tile([TILE, ctot], F32)
        nc.sync.dma_start(out=c_padm0[:], in_=padm[:, :])
        c_or0 = const.tile([TILE, nt], I32)
        nc.sync.dma_start(out=c_or0[:], in_=outrow[:, :])
        c_stats = const.tile([NLS + 1, S_], BF16)
        nc.sync.dma_start(out=c_stats[:], in_=statst[:, :])
        c_ab = const.tile([TILE, 1], F32)
        nc.sync.dma_start(out=c_ab[:], in_=ab_rep[:, :])
        c_id = const.tile([TILE, TILE], BF16)
        nc.sync.dma_start(out=c_id[:], in_=ident[:, :])
        c_w1a = const.tile([H, H], BF16)
        nc.sync.dma_start(out=c_w1a[:], in_=w1t_a[:, :])
        c_w1b = const.tile([NLS + 1, H], BF16)
        nc.sync.dma_start(out=c_w1b[:], in_=w1t_b[:, :])
        c_b1 = const.tile([H, 1], F32)
        nc.sync.dma_start(out=c_b1[:], in_=b1[:, :])
        c_w2 = const.tile([H, H], BF16)
        nc.sync.dma_start(out=c_w2[:], in_=w2t[:, :])
        c_b2 = const.tile([1, H], BF16)
        nc.sync.dma_start(out=c_b2[:], in_=b2row[:, :])
        c_ones = const.tile([1, TILE], BF16)
        nc.sync.dma_start(out=c_ones[:], in_=onesc[:, :])
        # a1/a2 replicated to 128 partitions (DMA broadcast)
        c_a2r0 = const.tile([TILE, H], BF16)
        nc.gpsimd.dma_start(out=c_a2r0[:], in_=bass.AP(
            tensor=a2rep_d.ap().tensor, offset=0, ap=[[0, TILE], [1, H]]))
        c_a1r0 = const.tile([TILE, H], BF16)
        nc.gpsimd.dma_start(out=c_a1r0[:], in_=bass.AP(
            tensor=a1rep_d.ap().tensor, offset=0, ap=[[0, TILE], [1, H]]))

        # ---- one-time fences: absorb const-DMA sems onto consuming engines
        # (steady-state ops may carry only one sync wait) ----
        c_a2r = const.tile([TILE, H], BF16)
        nc.vector.tensor_copy(out=c_a2r[:], in_=c_a2r0[:])
        c_a1r = const.tile([TILE, H], BF16)
        nc.vector.tensor_copy(out=c_a1r[:], in_=c_a1r0[:])
        c_idx = const.tile([TILE, ctot], I32)
        nc.vector.tensor_copy(out=c_idx[:], in_=c_idx0[:])
        c_padm = const.tile([TILE, ctot], F32)
        nc.vector.tensor_copy(out=c_padm[:], in_=c_padm0[:])
        c_or = const.tile([TILE, nt], I32)
        nc.vector.tensor_copy(out=c_or[:], in_=c_or0[:])
        c_ab2 = const.tile([TILE, 1], F32)
        nc.vector.tensor_copy(out=c_ab2[:], in_=c_ab[:])
        dpsum = psum.tile([TILE, TILE], F32, tag="dfence")
        nc.tensor.matmul(out=dpsum[:], lhsT=c_id[:], rhs=c_w1a[:], start=True, stop=True)
        nc.tensor.matmul(out=dpsum[:], lhsT=c_w2[:], rhs=c_id[:], start=True, stop=True)
        nc.tensor.matmul(
            out=dpsum[:], lhsT=c_w1b[:], rhs=c_stats[:, 0:TILE], start=True, stop=True)
        nc.tensor.matmul(out=dpsum[:], lhsT=c_ones[:], rhs=c_b2[:], start=True, stop=True)
        dact = const.tile([TILE, 1], F32)
        nc.scalar.activation(out=dact[:], in_=c_ab2[:], func=AF.Identity, bias=c_b1[:, 0:1])

        for rep in range(reps):
          for t in range(nt):
            ct = int(slots[t])
            o0 = int(offs[t])
            # ---- gather (f32 -> bf16 cast in flight), one call per slot ----
            g = gpool.tile([TILE, cmax * H], BF16, tag="g")
            for i in range(ct):
                nc.gpsimd.indirect_dma_start(
                    out=g[:, i * H:(i + 1) * H],
                    out_offset=None,
                    in_=emb.ap(),
                    in_offset=bass.IndirectOffsetOnAxis(
                        ap=c_idx[:, o0 + i:o0 + i + 1], axis=0),
                )

            # ---- scores: w[:, i] = g_i . a2  (broadcast-AP mul + seg reduce)
            t1 = spool.tile([TILE, cmax * H], BF16, tag="t1")
            a2b = bass.AP(tensor=c_a2r[:].tensor, offset=c_a2r[:].offset,
                          ap=[c_a2r[:].ap[0], [0, ct], [1, H]])
            nc.vector.tensor_tensor(
                out=t1[:, :ct * H].rearrange("p (i h) -> p i h", i=ct),
                in0=g[:, :ct * H].rearrange("p (i h) -> p i h", i=ct),
                in1=a2b, op=ALU.mult)
            w = small.tile([TILE, cmax], F32, tag="w")
            nc.vector.reduce_sum(
                out=w[:, :ct],
                in_=t1[:, :ct * H].rearrange("p (i h) -> p i h", i=ct),
                axis=mybir.AxisListType.X)
            # u = src . a1 (slot 0)
            t2 = small.tile([TILE, H], BF16, tag="t2")
            nc.vector.tensor_tensor(out=t2[:], in0=g[:, :H], in1=c_a1r[:], op=ALU.mult)
            u = small.tile([TILE, 1], F32, tag="u")
            nc.vector.reduce_sum(
                out=u[:], in_=t2[:].rearrange("p (i h) -> p i h", i=1),
                axis=mybir.AxisListType.X)
            # s = (w + u) + a_b, then leaky, then -1e9 on pad slots
            s = small.tile([TILE, cmax], F32, tag="s")
            nc.vector.tensor_scalar(
                out=s[:, :ct], in0=w[:, :ct], scalar1=u[:, 0:1],
                scalar2=c_ab2[:, 0:1], op0=ALU.add, op1=ALU.add)
            nc.vector.scalar_tensor_tensor(
                out=s[:, :ct], in0=s[:, :ct], scalar=0.2, in1=s[:, :ct],
                op0=ALU.mult, op1=ALU.max)
            nc.vector.scalar_tensor_tensor(
                out=s[:, :ct], in0=c_padm[:, o0:o0 + ct], scalar=-1e9,
                in1=s[:, :ct], op0=ALU.mult, op1=ALU.add)
            # softmax
            negm = small.tile([TILE, 1], F32, tag="negm")
            nc.vector.tensor_reduce(
                out=negm[:], in_=s[:, :ct], axis=mybir.AxisListType.X, op=ALU.max,
                negate=True)
            e = small.tile([TILE, cmax], F32, tag="e")
            zsum = small.tile([TILE, 1], F32, tag="zsum")
            nc.scalar.activation(
                out=e[:, :ct], in_=s[:, :ct], func=AF.Exp, bias=negm[:, 0:1],
                accum_out=zsum[:])
            r = small.tile([TILE, 1], F32, tag="r")
            nc.vector.reciprocal(out=r[:], in_=zsum[:])
            att = small.tile([TILE, cmax], F32, tag="att")
            nc.vector.tensor_scalar_mul(out=att[:, :ct], in0=e[:, :ct], scalar1=r[:, 0:1])

            # ---- attention prescale + PE transpose-accumulate ----
            gs = spool.tile([TILE, cmax * H], BF16, tag="gs")
            for i in range(ct):
                nc.vector.tensor_scalar_mul(
                    out=gs[:, i * H:(i + 1) * H], in0=g[:, i * H:(i + 1) * H],
                    scalar1=att[:, i:i + 1])
            vps = psum.tile([TILE, TILE], F32, tag="vps")
            for i in range(ct):
                nc.tensor.matmul(
                    out=vps[:], lhsT=gs[:, i * H:(i + 1) * H], rhs=c_id[:],
                    start=(i == 0), stop=(i == ct - 1))
            vt = small.tile([H, TILE], BF16, tag="vt")
            nc.scalar.activation(out=vt[:], in_=vps[:], func=AF.Copy)

            # ---- MLP head ----
            o1p = psum.tile([H, TILE], F32, tag="o1p")
            nc.tensor.matmul(out=o1p[:], lhsT=c_w1a[:], rhs=vt[:], start=True, stop=False)
            nc.tensor.matmul(
                out=o1p[:], lhsT=c_w1b[:], rhs=c_stats[:, t * TILE:(t + 1) * TILE],
                start=False, stop=True)
            o1c = small.tile([H, TILE], BF16, tag="o1c")
            nc.scalar.activation(out=o1c[:], in_=o1p[:], func=AF.Identity, bias=c_b1[:, 0:1])
            o1 = small.tile([H, TILE], BF16, tag="o1")
            nc.vector.scalar_tensor_tensor(
                out=o1[:], in0=o1c[:], scalar=0.2, in1=o1c[:], op0=ALU.mult, op1=ALU.max)
            o2p = psum.tile([TILE, H], F32, tag="o2p")
            nc.tensor.matmul(out=o2p[:], lhsT=o1[:], rhs=c_w2[:], start=True, stop=False)
            nc.tensor.matmul(out=o2p[:], lhsT=c_ones[:], rhs=c_b2[:], start=False, stop=True)
            otc = small.tile([TILE, H], F32, tag="otc")
            nc.scalar.activation(out=otc[:], in_=o2p[:], func=AF.Copy)
            ot = opool.tile([TILE, H], F32, tag="ot")
            nc.vector.scalar_tensor_tensor(
                out=ot[:], in0=otc[:], scalar=0.2, in1=otc[:], op0=ALU.mult, op1=ALU.max)
            # scatter-store to original node rows
            nc.gpsimd.indirect_dma_start(
                out=out.ap(),
                out_offset=bass.IndirectOffsetOnAxis(ap=c_or[:, t:t + 1], axis=0),
                in_=ot[:],
                in_offset=None)

    nc.finalize()
    return nc


def _prep_inputs(subgraph, neighs, mask, local_stats, global_stats,
                 emb_table, a_w, a_b, fc1_w, fc1_b, fc2_w, fc2_b):
    """Host-side layout/sharding prep. Returns (in_maps, slots)."""
    bf = ml_dtypes.bfloat16
    a1 = a_w[0, :H]
    a2 = a_w[0, H:]
    shared = {
        "emb": np.ascontiguousarray(emb_table, dtype=np.float32),
        "a2rep": a2.reshape(1, H).astype(bf),
        "a1rep": a1.reshape(1, H).astype(bf),
        "ab_rep": np.broadcast_to(a_b.astype(np.float32), (TILE, 1)).copy(),
        "ident": np.eye(TILE, dtype=np.float32).astype(bf),
        "w1t_a": np.ascontiguousarray(fc1_w[:, :H].T).astype(bf),
        "w1t_b": np.ascontiguousarray(fc1_w[:, H:].T).astype(bf),
        "b1": fc1_b.reshape(H, 1).astype(np.float32),
        "w2t": np.ascontiguousarray(fc2_w.T).astype(bf),
        "b2row": fc2_b.reshape(1, H).astype(bf),
        "onesc": np.ones((1, TILE), dtype=np.float32).astype(bf),
    }
    keep = mask[:, :, :, 0] < 0.5          # [B,S,N] True = neighbor survives
    counts = 1 + keep.sum(axis=2)          # [B,S] slots per node (src + kept)
    orders = np.argsort(-counts, axis=1, kind="stable")  # per-core node order

    # per-tile slot count = max over cores of the tile's max count (SPMD)
    slots = []
    for t in range(NT):
        c = 0
        for b in range(B):
            c = max(c, int(counts[b, orders[b, t * TILE]]))
        slots.append(c)
    slots = tuple(slots)
    offs = np.concatenate([[0], np.cumsum(slots)]).astype(int)
    ctot = int(offs[-1])

    in_maps = []
    for b in range(B):
        order = orders[b]
        idx = np.zeros((TILE, ctot), dtype=np.int32)
        padm = np.zeros((TILE, ctot), dtype=np.float32)
        outrow = np.zeros((TILE, NT), dtype=np.int32)
        for t in range(NT):
            ct = slots[t]
            o0 = offs[t]
            nodes = order[t * TILE:(t + 1) * TILE]
            outrow[:, t] = nodes
            idx[:, o0] = subgraph[b, nodes]
            for p in range(TILE):
                n = nodes[p]
                kn = neighs[b, n][keep[b, n]]
                idx[p, o0 + 1:o0 + 1 + len(kn)] = kn
                padm[p, o0 + 1 + len(kn):o0 + ct] = 1.0
        st = np.concatenate(
            [local_stats[b][order].T,
             np.broadcast_to(global_stats[b].reshape(1, 1), (1, S))], axis=0)
        m = dict(shared)
        m.update({
            "idx": idx, "padm": padm, "outrow": outrow,
            "statst": np.ascontiguousarray(st).astype(bf),
        })
        in_maps.append(m)
    return in_maps, slots


last_exec_ns = None
last_results = None


def kernel(**inputs) -> np.ndarray:
    global last_exec_ns, last_results
    in_maps, slots = _prep_inputs(**inputs)
    if slots not in _cached:
        _cached[slots] = _build_program(slots)
    nc = _cached[slots]
    trace = bool(int(os.environ.get("KERNEL_TRACE", "0")))
    res = bass_utils.run_bass_kernel_spmd(
        nc, in_maps, core_ids=list(range(B)), trace=trace)
    last_exec_ns = res.exec_time_ns
    last_results = res
    out = np.stack([res.results[b]["out"] for b in range(B)], axis=0)
    return out.astype(np.float32)


if __name__ == "__main__":
    _build_program((33,) * NT)
    print("program builds OK")

